# revision 48
# baseline (speedup 1.0000x reference)
"""Trainium2 Bass kernel for DifferentialAttention (B=2, S=2048, DIM=2048).

Sharding: 8 cores = 2 batches x 4 head-groups (4 heads each). Per core:
  - QKV projection (column-parallel slices of wq/wk/wv) + RoPE on device
  - differential attention for its 4 heads
  - row-parallel wo partial product; host sums the 4 partials per batch.

Key structural facts used:
  * reference's _repeat_kv_buggy maps head h=2*kv+r to kv-head `kv` with each
    source position in [r*1024,(r+1)*1024) duplicated twice. Duplicated keys
    cancel between softmax numerator and denominator, so head h attends to
    exactly the 1024 unique keys/values of its half -> half the attention work.
  * softmax without max-subtraction is safe here (|scores*scale| small).
  * RMSNorm folding: attn = u/d1 with u = pv1 - lam*(d1/d2)*pv2 gives
    normed = u * rsqrt(mean_dv(u^2) + eps*d1^2)  (no per-element division).
  * subln_w * (1-lambda_init) is folded into wo rows on the host.

Per-core layouts (partition dim first):
  QT [128,4,S]: q-head tiles; rows within a tile: [E0 O0 E1 O1] (branch-major,
     each branch's 64 dims permuted evens-first so RoPE pairs are 32-blocks).
  KT [128,2,S]: same for the 2 kv heads.
  Vn [128,16,256]: v in natural [s,dv] layout, s-tile major.
  scoresT: [keys 128, queries] so softmax denom / pv contract over partitions.
"""

import numpy as np
import ml_dtypes
import concourse.bass as bass
import concourse.tile as tile
from concourse import bacc, mybir
from concourse.bass_utils import run_bass_kernel_spmd
from contextlib import ExitStack

F32 = mybir.dt.float32
BF16 = mybir.dt.bfloat16
AF = mybir.ActivationFunctionType
ALU = mybir.AluOpType

DIM = 2048
S = 2048
B = 2
HD = 64          # rope head dim
EPS = 1e-5
SCALE = HD ** -0.5
NCORES = 8

BF = True              # bf16 matmul inputs (proj, scores, pv); psum stays fp32
STAGES = "abc"         # which stages to emit (timing experiments)
WO = True              # emit the wo matmul part of stage C
SQW = 1024             # stage-A s-chunk width (512 or 1024)
PVBF = True            # store pv accumulators in bf16 (halves SBUF)
WOBF = True            # bf16 attnT/wo operands for the output matmul
TRACE = False          # set by test.py to collect an NTFF profile
LAST_RESULTS = None    # BassKernelResults of the last run (for test.py)


def _dt():
    return BF16 if BF else F32


def _npdt():
    return ml_dtypes.bfloat16 if BF else np.float32


# ---------------------------------------------------------------- device program

def build_program(lam: float):
    nc = bacc.Bacc("TRN2", target_bir_lowering=False, debug=False,
                   num_devices=NCORES)
    dt = _dt()
    io = {}
    for name, shape, d in [
        ("xT", [DIM, S], dt), ("wq_s", [DIM, 512], dt), ("wk_s", [DIM, 256], dt),
        ("wv_s", [DIM, 256], dt), ("wo_s", [512, DIM], BF16 if WOBF else F32),
        ("cs128", [128, S], dt), ("sn128", [128, S], dt),
        ("ones_d", [128, 1], dt), ("ones_m", [128, 1], F32),
        ("ones_b", [1, 128], F32),
    ]:
        io[name] = nc.dram_tensor(name, shape, d, kind="ExternalInput").ap()
    out = nc.dram_tensor("out", [S, DIM], F32, kind="ExternalOutput").ap()

    with tile.TileContext(nc) as tc:
        _body(tc, io, out, lam)
    nc.compile()
    return nc


def _body(tc, io, out, lam):
    nc = tc.nc
    with ExitStack() as top:
        stash = top.enter_context(tc.tile_pool(name="stash", bufs=1))
        pvs = stash.tile([128, 8, S], BF16 if PVBF else F32)  # pv accums (j*4+h)
        brow = stash.tile([4, S], F32)      # eps*d1^2 rows
        rr = stash.tile([4, S], F32)        # d1/d2 rows
        mf = stash.tile([4, S], F32)        # mean(u^2) rows -> rs rows
        ones_d = stash.tile([128, 1], _dt())
        ones_m = stash.tile([128, 1], F32)
        ones_b = stash.tile([1, 128], F32)
        nc.sync.dma_start(ones_d[:], io["ones_d"][:])
        nc.sync.dma_start(ones_m[:], io["ones_m"][:])
        nc.sync.dma_start(ones_b[:], io["ones_b"][:])

        with ExitStack() as ab:
            qkvp = ab.enter_context(tc.tile_pool(name="qkvp", bufs=1))
            QT = qkvp.tile([128, 4, S], _dt())
            KT = qkvp.tile([128, 2, S], _dt())
            Vn = qkvp.tile([128, 16, 256], _dt())
            d1 = qkvp.tile([4, S], F32)
            d2 = qkvp.tile([4, S], F32)

            if "a" in STAGES:
                _stage_a(tc, io, QT, KT, Vn)
            if "b" in STAGES:
                _stage_b(tc, io, QT, KT, Vn, d1, d2, ones_d, pvs)
                # prologue rows (partitions 0..3, aligned):
                #   brow = eps*d1^2 ; rr = exp(ln d1 - ln d2) = d1/d2
                nc.vector.scalar_tensor_tensor(
                    brow[:], d1[:], float(EPS), d1[:],
                    op0=ALU.mult, op1=ALU.mult)
                nc.scalar.activation(d1[:], d1[:], AF.Ln, bias=0.0, scale=1.0)
                nc.scalar.activation(d2[:], d2[:], AF.Ln, bias=0.0, scale=1.0)
                nc.vector.tensor_sub(rr[:], d1[:], d2[:])
                nc.scalar.activation(rr[:], rr[:], AF.Exp, bias=0.0, scale=1.0)

        if "c" in STAGES:
            _stage_c(tc, io, out, pvs, brow, rr, mf, ones_m, ones_b, lam)


def _stage_a(tc, io, QT, KT, Vn):
    """QKV projection + RoPE. Loop s-halves; x^T half resident in SBUF."""
    nc = tc.nc
    dt = _dt()
    with ExitStack() as ctx:
        xp = ctx.enter_context(tc.tile_pool(name="xh", bufs=1))
        wp = ctx.enter_context(tc.tile_pool(name="wqk", bufs=2))
        wvp = ctx.enter_context(tc.tile_pool(name="wvp", bufs=1))
        trig = ctx.enter_context(tc.tile_pool(name="trig", bufs=1))
        tmp = ctx.enter_context(tc.tile_pool(name="ropetmp", bufs=2))
        ps = ctx.enter_context(tc.tile_pool(name="ps_qk", bufs=2, space="PSUM"))
        psv = ctx.enter_context(tc.tile_pool(name="ps_v", bufs=2, space="PSUM"))

        cs = trig.tile([128, S], _dt())
        sn = trig.tile([128, S], _dt())
        nc.sync.dma_start(cs[:], io["cs128"][:])
        nc.sync.dma_start(sn[:], io["sn128"][:])
        wv_all = wvp.tile([128, 16, 256], dt)
        nc.sync.dma_start(wv_all[:],
                          io["wv_s"].rearrange("(a p) c -> p a c", p=128))
        xT3 = io["xT"].rearrange("(a p) s -> p a s", p=128)
        wq3 = io["wq_s"].rearrange("(a p) c -> p a c", p=128)
        wk3 = io["wk_s"].rearrange("(a p) c -> p a c", p=128)

        W = SQW
        for sq in range(S // W):
            ssl = slice(sq * W, sq * W + W)
            xh = xp.tile([128, 16, W], dt, tag="xh")
            nc.sync.dma_start(xh[:], xT3[:, :, ssl])

            # --- Q (4 tiles) and K (2 tiles): out rows = head-dims, free = s
            for ct in range(6):
                wsrc, dest, di = (wq3, QT, ct) if ct < 4 else (wk3, KT, ct - 4)
                wct = wp.tile([128, 16, 128], dt, tag="w")
                nc.sync.dma_start(wct[:],
                                  wsrc[:, :, di * 128:(di + 1) * 128])
                pq = ps.tile([128, W], F32, tag="psqk")
                for dt_i in range(16):
                    for nch in range(W // 512):
                        nsl = slice(nch * 512, (nch + 1) * 512)
                        nc.tensor.matmul(pq[:, nsl], lhsT=wct[:, dt_i, :],
                                         rhs=xh[:, dt_i, nsl],
                                         start=(dt_i == 0), stop=(dt_i == 15))
                # RoPE: rows [E0 O0 E1 O1] x 32; row i of E/O block <-> freq i.
                # sn128 carries signs [+s;-s;+s;-s], so after swapping the
                # 32-row halves of t2 (via DMA, which may cross partitions)
                # the combine is a single base-aligned add:
                #   newE = E*cos + swap(O*(-sin)) ; newO = O*cos + swap(E*sin)
                t1 = tmp.tile([128, W], F32, tag="t1")
                t2 = tmp.tile([128, W], F32, tag="t2")
                t2s = tmp.tile([128, W], F32, tag="t2s")
                nc.vector.tensor_mul(t1[:], pq[:], cs[:, ssl])
                nc.vector.tensor_mul(t2[:], pq[:], sn[:, ssl])
                for blk in range(4):
                    a, bb = blk * 32, blk * 32 + 32
                    sw = (a + 32, bb + 32) if blk % 2 == 0 else (a - 32, bb - 32)
                    nc.sync.dma_start(t2s[a:bb, :], t2[sw[0]:sw[1], :])
                d3 = dest[:, di, ssl]
                nc.vector.tensor_add(d3[:], t1[:], t2s[:])

            # --- V natural layout [s, dv]: one psum per s-tile, accum over d
            for st in range(W // 128):
                pv = psv.tile([128, 256], F32, tag="psv")
                for dt_i in range(16):
                    nc.tensor.matmul(pv[:],
                                     lhsT=xh[:, dt_i, st * 128:(st + 1) * 128],
                                     rhs=wv_all[:, dt_i, :],
                                     start=(dt_i == 0), stop=(dt_i == 15))
                nc.vector.tensor_copy(Vn[:, sq * (W // 128) + st, :], pv[:])


def _stage_b(tc, io, QT, KT, Vn, d1, d2, ones_d, pvs):
    """Attention: per (head, branch, q-half): scoresT -> exp -> pv + denom."""
    nc = tc.nc
    with ExitStack() as ctx:
        ep = ctx.enter_context(tc.tile_pool(name="expt", bufs=4))
        pvb = ctx.enter_context(tc.tile_pool(name="pvbounce", bufs=2))
        pss = ctx.enter_context(tc.tile_pool(name="ps_s", bufs=2, space="PSUM"))
        psp = ctx.enter_context(tc.tile_pool(name="ps_pv", bufs=1, space="PSUM"))
        psd = ctx.enter_context(tc.tile_pool(name="ps_d", bufs=1, space="PSUM"))

        for h in range(4):
            kvl, rho = h // 2, h % 2
            for j in range(2):
                hb = j * 4 + h
                dflat = d1 if j == 0 else d2
                for qh in range(2):
                    qsl = slice(qh * 1024, qh * 1024 + 1024)
                    ppv = psp.tile([128, 1024], F32, tag="pv")
                    pd = psd.tile([1, 1024], F32, tag="d")
                    for kt in range(8):
                        psc = pss.tile([128, 1024], F32, tag="sc")
                        kof = rho * 1024 + kt * 128
                        for nch in range(2):
                            nsl = slice(nch * 512, (nch + 1) * 512)
                            qssl = slice(qh * 1024 + nch * 512,
                                         qh * 1024 + nch * 512 + 512)
                            nc.tensor.matmul(
                                psc[:, nsl],
                                lhsT=KT[j * 64:(j + 1) * 64, kvl,
                                        kof:kof + 128],
                                rhs=QT[j * 64:(j + 1) * 64, h, qssl],
                                start=True, stop=True)
                        et = ep.tile([128, 1024], _dt(), tag="e")
                        nc.scalar.activation(et[:], psc[:], AF.Exp,
                                             bias=0.0, scale=float(SCALE))
                        for nch in range(2):
                            nsl = slice(nch * 512, (nch + 1) * 512)
                            nc.tensor.matmul(
                                ppv[:, nsl],
                                lhsT=Vn[:, rho * 8 + kt,
                                        kvl * 128:(kvl + 1) * 128],
                                rhs=et[:, nsl],
                                start=(kt == 0), stop=(kt == 7))
                            nc.tensor.matmul(
                                pd[0:1, nsl], lhsT=ones_d[:], rhs=et[:, nsl],
                                start=(kt == 0), stop=(kt == 7))
                    # PSUM is not DMA-readable: evacuate via DVE
                    nc.vector.tensor_copy(pvs[:, hb, qsl], ppv[:])
                    db = pvb.tile([1, 1024], F32, tag="db")
                    nc.vector.tensor_copy(db[0:1, :], pd[0:1, :])
                    nc.sync.dma_start(dflat[h:h + 1, qsl], db[0:1, :])


def _stage_c(tc, io, out, pvs, brow, rr, mf, ones_m, ones_b, lam):
    """Combine branches, RMS-normalize (division-free), apply wo."""
    nc = tc.nc
    udt = BF16 if WOBF else F32
    with ExitStack() as ctx:
        usp = ctx.enter_context(tc.tile_pool(name="usp", bufs=1))
        uf = usp.tile([128, 8, 1024], F32)    # u (fp32, for u^2 and final mul)
        ust = usp.tile([128, 8, 1024], udt)   # attnT = u*rs (wo lhsT dtype)
        _norm_pass(tc, pvs, brow, rr, mf, ones_m, ones_b, lam, uf, ust)
        _wo_pass(tc, io, out, ust)


def _norm_pass(tc, pvs, brow, rr, mf, ones_m, ones_b, lam, uf, ust):
    nc = tc.nc
    with ExitStack() as ctx:
        tmp = ctx.enter_context(tc.tile_pool(name="ctmp", bufs=2))
        rsp = ctx.enter_context(tc.tile_pool(name="rstage", bufs=3))
        psb = ctx.enter_context(tc.tile_pool(name="ps_bc", bufs=2, space="PSUM"))
        psm = ctx.enter_context(tc.tile_pool(name="ps_m", bufs=1, space="PSUM"))
        # pass 1: build u, row-means of u^2
        for h in range(4):
            for qh in range(2):
                qsl = slice(qh * 1024, qh * 1024 + 1024)
                rs1 = rsp.tile([1, 1024], F32, tag="rowstage")
                nc.sync.dma_start(rs1[0:1, :], rr[h:h + 1, qsl])
                bc = psb.tile([128, 1024], F32, tag="bc")
                for nch in range(2):
                    nsl = slice(nch * 512, (nch + 1) * 512)
                    nc.tensor.matmul(bc[:, nsl], lhsT=ones_b[:],
                                     rhs=rs1[0:1, nsl], start=True, stop=True)
                t = tmp.tile([128, 1024], F32, tag="t")
                nc.vector.scalar_tensor_tensor(
                    t[:], bc[:], float(lam), pvs[:, 4 + h, qsl],
                    op0=ALU.mult, op1=ALU.mult)
                u = uf[:, h * 2 + qh, :]
                nc.vector.tensor_sub(u[:], pvs[:, h, qsl], t[:])
                sq = tmp.tile([128, 1024], F32, tag="sq")
                nc.vector.tensor_mul(sq[:], u[:], u[:])
                pm = psm.tile([1, 1024], F32, tag="m")
                for nch in range(2):
                    nsl = slice(nch * 512, (nch + 1) * 512)
                    nc.tensor.matmul(pm[0:1, nsl], lhsT=ones_m[:],
                                     rhs=sq[:, nsl], start=True, stop=True)
                mb = rsp.tile([1, 1024], F32, tag="rowstage")
                nc.vector.tensor_copy(mb[0:1, :], pm[0:1, :])
                nc.sync.dma_start(mf[h:h + 1, qsl], mb[0:1, :])

        # rs = rsqrt(mean(u^2) + eps*d1^2) = exp(-0.5*ln(mf + brow))
        nc.vector.tensor_add(mf[:], mf[:], brow[:])
        nc.scalar.activation(mf[:], mf[:], AF.Ln, bias=0.0, scale=1.0)
        nc.scalar.activation(mf[:], mf[:], AF.Exp, bias=0.0, scale=-0.5)

        # pass 2: attnT = u * bcast(rs)
        for h in range(4):
            for qh in range(2):
                qsl = slice(qh * 1024, qh * 1024 + 1024)
                rs2 = rsp.tile([1, 1024], F32, tag="rowstage")
                nc.sync.dma_start(rs2[0:1, :], mf[h:h + 1, qsl])
                bc2 = psb.tile([128, 1024], F32, tag="bc")
                for nch in range(2):
                    nsl = slice(nch * 512, (nch + 1) * 512)
                    nc.tensor.matmul(bc2[:, nsl], lhsT=ones_b[:],
                                     rhs=rs2[0:1, nsl], start=True, stop=True)
                idx = h * 2 + qh
                nc.vector.tensor_mul(ust[:, idx, :], uf[:, idx, :], bc2[:])


def _wo_pass(tc, io, out, ust):
    """out[s,e] = sum_r attnT[r,s] * wo_s[r,e]; lhsT reused across e-chunks."""
    nc = tc.nc
    if not WO:
        return
    wdt = BF16 if WOBF else F32
    with ExitStack() as ctx:
        wop = ctx.enter_context(tc.tile_pool(name="wop", bufs=1))
        obp = ctx.enter_context(tc.tile_pool(name="obp", bufs=2))
        pso = ctx.enter_context(tc.tile_pool(name="ps_o", bufs=2, space="PSUM"))
        wot = wop.tile([128, 4, S], wdt)
        nc.sync.dma_start(wot[:], io["wo_s"].rearrange("(a p) c -> p a c", p=128))
        for st in range(16):
            pos = [pso.tile([128, 512], F32, tag=f"o{e}", name=f"po{e}")
                   for e in range(4)]
            for r in range(4):
                lhsT = ust[:, r * 2 + st // 8, (st % 8) * 128:
                           (st % 8) * 128 + 128]
                for ech in range(4):
                    nc.tensor.matmul(pos[ech][:],
                                     lhsT=lhsT,
                                     rhs=wot[:, r, ech * 512:(ech + 1) * 512],
                                     start=(r == 0), stop=(r == 3))
            for ech in range(4):
                ob = obp.tile([128, 512], F32, tag="ob")
                nc.vector.tensor_copy(ob[:], pos[ech][:])
                nc.sync.dma_start(out[st * 128:(st + 1) * 128,
                                      ech * 512:(ech + 1) * 512], ob[:])


# ---------------------------------------------------------------- host side

_PERM64 = np.concatenate([np.arange(0, 64, 2), np.arange(1, 64, 2)])


def make_core_inputs(core, x, wq, wk, wv, wo, subln_w, lambda_init,
                     freqs_cos, freqs_sin):
    b, g = divmod(core, 4)
    npdt = _npdt()
    qcols = np.empty(512, np.int64)
    for hl in range(4):
        for j in range(2):
            qcols[hl * 128 + j * 64:hl * 128 + j * 64 + 64] = \
                ((4 * g + hl) * 2 + j) * 64 + _PERM64
    kcols = np.empty(256, np.int64)
    for kvl in range(2):
        for j in range(2):
            kcols[kvl * 128 + j * 64:kvl * 128 + j * 64 + 64] = \
                ((2 * g + kvl) * 2 + j) * 64 + _PERM64
    vcols = np.arange(256) + 2 * g * 128

    cosT = np.ascontiguousarray(freqs_cos.T.astype(np.float32))  # [32, S]
    sinT = np.ascontiguousarray(freqs_sin.T.astype(np.float32))
    wo_s = wo[512 * g: 512 * g + 512, :].astype(np.float32).copy()
    wo_s *= np.tile(subln_w.astype(np.float32)
                    * (1.0 - np.float32(np.asarray(lambda_init)[0])), 4)[:, None]
    return {
        "xT": np.ascontiguousarray(x[b].T.astype(np.float32)).astype(npdt),
        "wq_s": np.ascontiguousarray(wq[:, qcols].astype(np.float32)).astype(npdt),
        "wk_s": np.ascontiguousarray(wk[:, kcols].astype(np.float32)).astype(npdt),
        "wv_s": np.ascontiguousarray(wv[:, vcols].astype(np.float32)).astype(npdt),
        "wo_s": wo_s.astype(ml_dtypes.bfloat16 if WOBF else np.float32),
        "cs128": np.tile(cosT, (4, 1)).astype(npdt),
        "sn128": np.concatenate([sinT, -sinT, sinT, -sinT], axis=0).astype(npdt),
        "ones_d": np.ones((128, 1), npdt),
        "ones_m": np.full((128, 1), 1.0 / 128.0, np.float32),
        "ones_b": np.ones((1, 128), np.float32),
    }


def compute_lambda(lambda_q1, lambda_k1, lambda_q2, lambda_k2, lambda_init):
    l1 = np.exp(np.sum(np.float32(lambda_q1) * np.float32(lambda_k1),
                       dtype=np.float32))
    l2 = np.exp(np.sum(np.float32(lambda_q2) * np.float32(lambda_k2),
                       dtype=np.float32))
    return float(l1 - l2 + np.float32(np.asarray(lambda_init)[0]))


def kernel(x, wq, wk, wv, wo, lambda_q1, lambda_k1, lambda_q2, lambda_k2,
           lambda_init, subln_w, freqs_cos, freqs_sin):
    global LAST_RESULTS
    x = np.asarray(x); wq = np.asarray(wq); wk = np.asarray(wk)
    wv = np.asarray(wv); wo = np.asarray(wo)
    lam = compute_lambda(lambda_q1, lambda_k1, lambda_q2, lambda_k2, lambda_init)

    nc = build_program(lam)
    in_maps = [make_core_inputs(c, x, wq, wk, wv, wo,
                                np.asarray(subln_w), np.asarray(lambda_init),
                                np.asarray(freqs_cos), np.asarray(freqs_sin))
               for c in range(NCORES)]
    res = run_bass_kernel_spmd(nc, in_maps, list(range(NCORES)), trace=TRACE)
    LAST_RESULTS = res
    outs = [res.results[c]["out"] for c in range(NCORES)]
    full = np.empty((B, S, DIM), np.float32)
    for b in range(B):
        full[b] = outs[4 * b] + outs[4 * b + 1] + outs[4 * b + 2] + outs[4 * b + 3]
    return full


# revision 49
# speedup vs baseline: 1.0701x; 1.0701x over previous
"""Trainium2 Bass kernel for DifferentialAttention (B=2, S=2048, DIM=2048).

Sharding: 8 cores = 2 batches x 4 head-groups (4 heads each). Per core:
  - QKV projection (column-parallel slices of wq/wk/wv) + RoPE on device
  - differential attention for its 4 heads
  - row-parallel wo partial product; host sums the 4 partials per batch.

Key structural facts used:
  * reference's _repeat_kv_buggy maps head h=2*kv+r to kv-head `kv` with each
    source position in [r*1024,(r+1)*1024) duplicated twice. Duplicated keys
    cancel between softmax numerator and denominator, so head h attends to
    exactly the 1024 unique keys/values of its half -> half the attention work.
  * softmax without max-subtraction is safe here (|scores*scale| small).
  * RMSNorm folding: attn = u/d1 with u = pv1 - lam*(d1/d2)*pv2 gives
    normed = u * rsqrt(mean_dv(u^2) + eps*d1^2)  (no per-element division).
  * subln_w * (1-lambda_init) is folded into wo rows on the host.

Per-core layouts (partition dim first):
  QT [128,4,S]: q-head tiles; rows within a tile: [E0 O0 E1 O1] (branch-major,
     each branch's 64 dims permuted evens-first so RoPE pairs are 32-blocks).
  KT [128,2,S]: same for the 2 kv heads.
  Vn [128,16,256]: v in natural [s,dv] layout, s-tile major.
  scoresT: [keys 128, queries] so softmax denom / pv contract over partitions.
"""

import numpy as np
import ml_dtypes
import concourse.bass as bass
import concourse.tile as tile
from concourse import bacc, mybir
from concourse.bass_utils import run_bass_kernel_spmd
from contextlib import ExitStack

F32 = mybir.dt.float32
BF16 = mybir.dt.bfloat16
AF = mybir.ActivationFunctionType
ALU = mybir.AluOpType

DIM = 2048
S = 2048
B = 2
HD = 64          # rope head dim
EPS = 1e-5
SCALE = HD ** -0.5
NCORES = 8

BF = True              # bf16 matmul inputs (proj, scores, pv); psum stays fp32
STAGES = "abc"         # which stages to emit (timing experiments)
WO = True              # emit the wo matmul part of stage C
SQW = 1024             # stage-A s-chunk width (512 or 1024)
PVBF = True            # store pv accumulators in bf16 (halves SBUF)
WOBF = True            # bf16 attnT/wo operands for the output matmul
TRACE = False          # set by test.py to collect an NTFF profile
LAST_RESULTS = None    # BassKernelResults of the last run (for test.py)


def _dt():
    return BF16 if BF else F32


def _npdt():
    return ml_dtypes.bfloat16 if BF else np.float32


# ---------------------------------------------------------------- device program

def build_program(lam: float):
    nc = bacc.Bacc("TRN2", target_bir_lowering=False, debug=False,
                   num_devices=NCORES)
    dt = _dt()
    io = {}
    for name, shape, d in [
        ("xT", [DIM, S], dt), ("wq_s", [DIM, 512], dt), ("wk_s", [DIM, 256], dt),
        ("wv_s", [DIM, 256], dt), ("wo_s", [512, DIM], BF16 if WOBF else F32),
        ("cs128", [128, S], dt), ("sn128", [128, S], dt),
        ("ones_d", [128, 1], dt), ("ones_m", [128, 1], F32),
        ("ones_b", [1, 128], F32),
    ]:
        io[name] = nc.dram_tensor(name, shape, d, kind="ExternalInput").ap()
    out = nc.dram_tensor("out", [S, DIM], F32, kind="ExternalOutput").ap()

    with tile.TileContext(nc) as tc:
        _body(tc, io, out, lam)
    nc.compile()
    return nc


def _body(tc, io, out, lam):
    nc = tc.nc
    with ExitStack() as top:
        stash = top.enter_context(tc.tile_pool(name="stash", bufs=1))
        pvs = stash.tile([128, 8, S], BF16 if PVBF else F32)  # pv accums (j*4+h)
        brow = stash.tile([4, S], F32)      # eps*d1^2 rows
        rr = stash.tile([4, S], F32)        # d1/d2 rows
        mf = stash.tile([4, S], F32)        # mean(u^2) rows -> rs rows
        ones_d = stash.tile([128, 1], _dt())
        ones_m = stash.tile([128, 1], F32)
        ones_b = stash.tile([1, 128], F32)
        nc.sync.dma_start(ones_d[:], io["ones_d"][:])
        nc.sync.dma_start(ones_m[:], io["ones_m"][:])
        nc.sync.dma_start(ones_b[:], io["ones_b"][:])

        with ExitStack() as ab:
            qkvp = ab.enter_context(tc.tile_pool(name="qkvp", bufs=1))
            QT = qkvp.tile([128, 4, S], _dt())
            KT = qkvp.tile([128, 2, S], _dt())
            Vn = qkvp.tile([128, 16, 256], _dt())
            d1 = qkvp.tile([4, S], F32)
            d2 = qkvp.tile([4, S], F32)

            if "a" in STAGES:
                _stage_a(tc, io, QT, KT, Vn)
            if "b" in STAGES:
                _stage_b(tc, io, QT, KT, Vn, d1, d2, ones_d, pvs)
                # prologue rows (partitions 0..3, aligned):
                #   brow = eps*d1^2 ; rr = exp(ln d1 - ln d2) = d1/d2
                nc.vector.scalar_tensor_tensor(
                    brow[:], d1[:], float(EPS), d1[:],
                    op0=ALU.mult, op1=ALU.mult)
                nc.scalar.activation(d1[:], d1[:], AF.Ln, bias=0.0, scale=1.0)
                nc.scalar.activation(d2[:], d2[:], AF.Ln, bias=0.0, scale=1.0)
                nc.vector.tensor_sub(rr[:], d1[:], d2[:])
                nc.scalar.activation(rr[:], rr[:], AF.Exp, bias=0.0, scale=1.0)

        if "c" in STAGES:
            _stage_c(tc, io, out, pvs, brow, rr, mf, ones_m, ones_b, lam)


def _stage_a(tc, io, QT, KT, Vn):
    """QKV projection + RoPE. Loop s-halves; x^T half resident in SBUF."""
    nc = tc.nc
    dt = _dt()
    with ExitStack() as ctx:
        xp = ctx.enter_context(tc.tile_pool(name="xh", bufs=1))
        wp = ctx.enter_context(tc.tile_pool(name="wqk", bufs=2))
        wvp = ctx.enter_context(tc.tile_pool(name="wvp", bufs=1))
        trig = ctx.enter_context(tc.tile_pool(name="trig", bufs=1))
        tmp = ctx.enter_context(tc.tile_pool(name="ropetmp", bufs=2))
        ps = ctx.enter_context(tc.tile_pool(name="ps_qk", bufs=2, space="PSUM"))
        psv = ctx.enter_context(tc.tile_pool(name="ps_v", bufs=2, space="PSUM"))

        cs = trig.tile([128, S], _dt())
        sn = trig.tile([128, S], _dt())
        nc.sync.dma_start(cs[:], io["cs128"][:])
        nc.sync.dma_start(sn[:], io["sn128"][:])
        wv_all = wvp.tile([128, 16, 256], dt)
        nc.sync.dma_start(wv_all[:],
                          io["wv_s"].rearrange("(a p) c -> p a c", p=128))
        xT3 = io["xT"].rearrange("(a p) s -> p a s", p=128)
        wq3 = io["wq_s"].rearrange("(a p) c -> p a c", p=128)
        wk3 = io["wk_s"].rearrange("(a p) c -> p a c", p=128)

        W = SQW
        for sq in range(S // W):
            ssl = slice(sq * W, sq * W + W)
            xh = xp.tile([128, 16, W], dt, tag="xh")
            nc.sync.dma_start(xh[:], xT3[:, :, ssl])

            # --- Q (4 tiles) and K (2 tiles): out rows = head-dims, free = s
            for ct in range(6):
                wsrc, dest, di = (wq3, QT, ct) if ct < 4 else (wk3, KT, ct - 4)
                wct = wp.tile([128, 16, 128], dt, tag="w")
                nc.sync.dma_start(wct[:],
                                  wsrc[:, :, di * 128:(di + 1) * 128])
                pq = ps.tile([128, W], F32, tag="psqk")
                for dt_i in range(16):
                    for nch in range(W // 512):
                        nsl = slice(nch * 512, (nch + 1) * 512)
                        nc.tensor.matmul(pq[:, nsl], lhsT=wct[:, dt_i, :],
                                         rhs=xh[:, dt_i, nsl],
                                         start=(dt_i == 0), stop=(dt_i == 15))
                # RoPE: rows [E0 O0 E1 O1] x 32; row i of E/O block <-> freq i.
                # sn128 carries signs [+s;-s;+s;-s], so after swapping the
                # 32-row halves of t2 (via DMA, which may cross partitions)
                # the combine is a single base-aligned add:
                #   newE = E*cos + swap(O*(-sin)) ; newO = O*cos + swap(E*sin)
                t1 = tmp.tile([128, W], F32, tag="t1")
                t2 = tmp.tile([128, W], F32, tag="t2")
                t2s = tmp.tile([128, W], F32, tag="t2s")
                nc.vector.tensor_mul(t1[:], pq[:], cs[:, ssl])
                nc.vector.tensor_mul(t2[:], pq[:], sn[:, ssl])
                for blk in range(4):
                    a, bb = blk * 32, blk * 32 + 32
                    sw = (a + 32, bb + 32) if blk % 2 == 0 else (a - 32, bb - 32)
                    nc.sync.dma_start(t2s[a:bb, :], t2[sw[0]:sw[1], :])
                d3 = dest[:, di, ssl]
                nc.vector.tensor_add(d3[:], t1[:], t2s[:])

            # --- V natural layout [s, dv]: one psum per s-tile, accum over d
            for st in range(W // 128):
                pv = psv.tile([128, 256], F32, tag="psv")
                for dt_i in range(16):
                    nc.tensor.matmul(pv[:],
                                     lhsT=xh[:, dt_i, st * 128:(st + 1) * 128],
                                     rhs=wv_all[:, dt_i, :],
                                     start=(dt_i == 0), stop=(dt_i == 15))
                nc.vector.tensor_copy(Vn[:, sq * (W // 128) + st, :], pv[:])


def _stage_b(tc, io, QT, KT, Vn, d1, d2, ones_d, pvs):
    """Attention: per (head, branch, q-half): scoresT -> exp -> pv + denom."""
    nc = tc.nc
    with ExitStack() as ctx:
        ep = ctx.enter_context(tc.tile_pool(name="expt", bufs=4))
        pvb = ctx.enter_context(tc.tile_pool(name="pvbounce", bufs=2))
        pss = ctx.enter_context(tc.tile_pool(name="ps_s", bufs=2, space="PSUM"))
        psp = ctx.enter_context(tc.tile_pool(name="ps_pv", bufs=1, space="PSUM"))
        psd = ctx.enter_context(tc.tile_pool(name="ps_d", bufs=1, space="PSUM"))

        for h in range(4):
            kvl, rho = h // 2, h % 2
            for j in range(2):
                hb = j * 4 + h
                dflat = d1 if j == 0 else d2
                for qh in range(2):
                    qsl = slice(qh * 1024, qh * 1024 + 1024)
                    ppv = psp.tile([128, 1024], F32, tag="pv")
                    pd = psd.tile([1, 1024], F32, tag="d")
                    for kt in range(8):
                        psc = pss.tile([128, 1024], F32, tag="sc")
                        kof = rho * 1024 + kt * 128
                        for nch in range(2):
                            nsl = slice(nch * 512, (nch + 1) * 512)
                            qssl = slice(qh * 1024 + nch * 512,
                                         qh * 1024 + nch * 512 + 512)
                            nc.tensor.matmul(
                                psc[:, nsl],
                                lhsT=KT[j * 64:(j + 1) * 64, kvl,
                                        kof:kof + 128],
                                rhs=QT[j * 64:(j + 1) * 64, h, qssl],
                                start=True, stop=True)
                        et = ep.tile([128, 1024], _dt(), tag="e")
                        nc.scalar.activation(et[:], psc[:], AF.Exp,
                                             bias=0.0, scale=float(SCALE))
                        for nch in range(2):
                            nsl = slice(nch * 512, (nch + 1) * 512)
                            nc.tensor.matmul(
                                ppv[:, nsl],
                                lhsT=Vn[:, rho * 8 + kt,
                                        kvl * 128:(kvl + 1) * 128],
                                rhs=et[:, nsl],
                                start=(kt == 0), stop=(kt == 7))
                            nc.tensor.matmul(
                                pd[0:1, nsl], lhsT=ones_d[:], rhs=et[:, nsl],
                                start=(kt == 0), stop=(kt == 7))
                    # PSUM is not DMA-readable: evacuate via DVE
                    nc.vector.tensor_copy(pvs[:, hb, qsl], ppv[:])
                    db = pvb.tile([1, 1024], F32, tag="db")
                    nc.vector.tensor_copy(db[0:1, :], pd[0:1, :])
                    nc.sync.dma_start(dflat[h:h + 1, qsl], db[0:1, :])


def _stage_c(tc, io, out, pvs, brow, rr, mf, ones_m, ones_b, lam):
    """Combine branches, RMS-normalize (division-free), apply wo."""
    nc = tc.nc
    udt = BF16 if WOBF else F32
    with ExitStack() as ctx:
        usp = ctx.enter_context(tc.tile_pool(name="usp", bufs=1))
        uf = usp.tile([128, 8, 1024], F32)    # u (fp32, for u^2 and final mul)
        ust = usp.tile([128, 8, 1024], udt)   # attnT = u*rs (wo lhsT dtype)
        _norm_pass(tc, pvs, brow, rr, mf, ones_m, ones_b, lam, uf, ust)
        _wo_pass(tc, io, out, ust)


def _norm_pass(tc, pvs, brow, rr, mf, ones_m, ones_b, lam, uf, ust):
    nc = tc.nc
    with ExitStack() as ctx:
        tmp = ctx.enter_context(tc.tile_pool(name="ctmp", bufs=2))
        rsp = ctx.enter_context(tc.tile_pool(name="rstage", bufs=3))
        psb = ctx.enter_context(tc.tile_pool(name="ps_bc", bufs=2, space="PSUM"))
        psm = ctx.enter_context(tc.tile_pool(name="ps_m", bufs=1, space="PSUM"))
        # pass 1: build u, row-means of u^2
        for h in range(4):
            for qh in range(2):
                qsl = slice(qh * 1024, qh * 1024 + 1024)
                rs1 = rsp.tile([1, 1024], F32, tag="rowstage")
                nc.sync.dma_start(rs1[0:1, :], rr[h:h + 1, qsl])
                bc = psb.tile([128, 1024], F32, tag="bc")
                for nch in range(2):
                    nsl = slice(nch * 512, (nch + 1) * 512)
                    nc.tensor.matmul(bc[:, nsl], lhsT=ones_b[:],
                                     rhs=rs1[0:1, nsl], start=True, stop=True)
                t = tmp.tile([128, 1024], F32, tag="t")
                nc.vector.scalar_tensor_tensor(
                    t[:], bc[:], float(lam), pvs[:, 4 + h, qsl],
                    op0=ALU.mult, op1=ALU.mult)
                u = uf[:, h * 2 + qh, :]
                nc.vector.tensor_sub(u[:], pvs[:, h, qsl], t[:])
                sq = tmp.tile([128, 1024], F32, tag="sq")
                nc.scalar.square(sq[:], u[:])
                pm = psm.tile([1, 1024], F32, tag="m")
                for nch in range(2):
                    nsl = slice(nch * 512, (nch + 1) * 512)
                    nc.tensor.matmul(pm[0:1, nsl], lhsT=ones_m[:],
                                     rhs=sq[:, nsl], start=True, stop=True)
                mb = rsp.tile([1, 1024], F32, tag="rowstage")
                nc.vector.tensor_copy(mb[0:1, :], pm[0:1, :])
                nc.sync.dma_start(mf[h:h + 1, qsl], mb[0:1, :])

        # rs = rsqrt(mean(u^2) + eps*d1^2) = exp(-0.5*ln(mf + brow))
        nc.vector.tensor_add(mf[:], mf[:], brow[:])
        nc.scalar.activation(mf[:], mf[:], AF.Ln, bias=0.0, scale=1.0)
        nc.scalar.activation(mf[:], mf[:], AF.Exp, bias=0.0, scale=-0.5)

        # pass 2: attnT = u * bcast(rs)
        for h in range(4):
            for qh in range(2):
                qsl = slice(qh * 1024, qh * 1024 + 1024)
                rs2 = rsp.tile([1, 1024], F32, tag="rowstage")
                nc.sync.dma_start(rs2[0:1, :], mf[h:h + 1, qsl])
                bc2 = psb.tile([128, 1024], F32, tag="bc")
                for nch in range(2):
                    nsl = slice(nch * 512, (nch + 1) * 512)
                    nc.tensor.matmul(bc2[:, nsl], lhsT=ones_b[:],
                                     rhs=rs2[0:1, nsl], start=True, stop=True)
                idx = h * 2 + qh
                nc.vector.tensor_mul(ust[:, idx, :], uf[:, idx, :], bc2[:])


def _wo_pass(tc, io, out, ust):
    """out[s,e] = sum_r attnT[r,s] * wo_s[r,e]; lhsT reused across e-chunks."""
    nc = tc.nc
    if not WO:
        return
    wdt = BF16 if WOBF else F32
    with ExitStack() as ctx:
        wop = ctx.enter_context(tc.tile_pool(name="wop", bufs=1))
        obp = ctx.enter_context(tc.tile_pool(name="obp", bufs=2))
        pso = ctx.enter_context(tc.tile_pool(name="ps_o", bufs=2, space="PSUM"))
        wot = wop.tile([128, 4, S], wdt)
        nc.sync.dma_start(wot[:], io["wo_s"].rearrange("(a p) c -> p a c", p=128))
        for st in range(16):
            pos = [pso.tile([128, 512], F32, tag=f"o{e}", name=f"po{e}")
                   for e in range(4)]
            for r in range(4):
                lhsT = ust[:, r * 2 + st // 8, (st % 8) * 128:
                           (st % 8) * 128 + 128]
                for ech in range(4):
                    nc.tensor.matmul(pos[ech][:],
                                     lhsT=lhsT,
                                     rhs=wot[:, r, ech * 512:(ech + 1) * 512],
                                     start=(r == 0), stop=(r == 3))
            for ech in range(4):
                ob = obp.tile([128, 512], F32, tag="ob")
                nc.vector.tensor_copy(ob[:], pos[ech][:])
                nc.sync.dma_start(out[st * 128:(st + 1) * 128,
                                      ech * 512:(ech + 1) * 512], ob[:])


# ---------------------------------------------------------------- host side

_PERM64 = np.concatenate([np.arange(0, 64, 2), np.arange(1, 64, 2)])


def make_core_inputs(core, x, wq, wk, wv, wo, subln_w, lambda_init,
                     freqs_cos, freqs_sin):
    b, g = divmod(core, 4)
    npdt = _npdt()
    qcols = np.empty(512, np.int64)
    for hl in range(4):
        for j in range(2):
            qcols[hl * 128 + j * 64:hl * 128 + j * 64 + 64] = \
                ((4 * g + hl) * 2 + j) * 64 + _PERM64
    kcols = np.empty(256, np.int64)
    for kvl in range(2):
        for j in range(2):
            kcols[kvl * 128 + j * 64:kvl * 128 + j * 64 + 64] = \
                ((2 * g + kvl) * 2 + j) * 64 + _PERM64
    vcols = np.arange(256) + 2 * g * 128

    cosT = np.ascontiguousarray(freqs_cos.T.astype(np.float32))  # [32, S]
    sinT = np.ascontiguousarray(freqs_sin.T.astype(np.float32))
    wo_s = wo[512 * g: 512 * g + 512, :].astype(np.float32).copy()
    wo_s *= np.tile(subln_w.astype(np.float32)
                    * (1.0 - np.float32(np.asarray(lambda_init)[0])), 4)[:, None]
    return {
        "xT": np.ascontiguousarray(x[b].T.astype(np.float32)).astype(npdt),
        "wq_s": np.ascontiguousarray(wq[:, qcols].astype(np.float32)).astype(npdt),
        "wk_s": np.ascontiguousarray(wk[:, kcols].astype(np.float32)).astype(npdt),
        "wv_s": np.ascontiguousarray(wv[:, vcols].astype(np.float32)).astype(npdt),
        "wo_s": wo_s.astype(ml_dtypes.bfloat16 if WOBF else np.float32),
        "cs128": np.tile(cosT, (4, 1)).astype(npdt),
        "sn128": np.concatenate([sinT, -sinT, sinT, -sinT], axis=0).astype(npdt),
        "ones_d": np.ones((128, 1), npdt),
        "ones_m": np.full((128, 1), 1.0 / 128.0, np.float32),
        "ones_b": np.ones((1, 128), np.float32),
    }


def compute_lambda(lambda_q1, lambda_k1, lambda_q2, lambda_k2, lambda_init):
    l1 = np.exp(np.sum(np.float32(lambda_q1) * np.float32(lambda_k1),
                       dtype=np.float32))
    l2 = np.exp(np.sum(np.float32(lambda_q2) * np.float32(lambda_k2),
                       dtype=np.float32))
    return float(l1 - l2 + np.float32(np.asarray(lambda_init)[0]))


def kernel(x, wq, wk, wv, wo, lambda_q1, lambda_k1, lambda_q2, lambda_k2,
           lambda_init, subln_w, freqs_cos, freqs_sin):
    global LAST_RESULTS
    x = np.asarray(x); wq = np.asarray(wq); wk = np.asarray(wk)
    wv = np.asarray(wv); wo = np.asarray(wo)
    lam = compute_lambda(lambda_q1, lambda_k1, lambda_q2, lambda_k2, lambda_init)

    nc = build_program(lam)
    in_maps = [make_core_inputs(c, x, wq, wk, wv, wo,
                                np.asarray(subln_w), np.asarray(lambda_init),
                                np.asarray(freqs_cos), np.asarray(freqs_sin))
               for c in range(NCORES)]
    res = run_bass_kernel_spmd(nc, in_maps, list(range(NCORES)), trace=TRACE)
    LAST_RESULTS = res
    outs = [res.results[c]["out"] for c in range(NCORES)]
    full = np.empty((B, S, DIM), np.float32)
    for b in range(B):
        full[b] = outs[4 * b] + outs[4 * b + 1] + outs[4 * b + 2] + outs[4 * b + 3]
    return full
